# revision 5
# baseline (speedup 1.0000x reference)
"""v9 (CHUNK=1024, 6-deep): host packs the five inputs into one per-chunk contiguous block
[m|fd|bd|fvx|fvy|bvx|bvy] so each chunk is ONE 7.3MB DMA with 56KB
contiguous runs (vs five transfers with 8-16KB runs), and the vote-dots
read dense step-1 de-interleaved vector components. Compute structure:
STT-family products, ping-pong dead-outs, ACT msum to PSUM (6 STT dots + 2 TT products, SBUF
dead-outs => 2x-mode eligible on fp32 tensor-scalar family; ACT msum to
PSUM), with DMA-side polish:
  - m/fd/bd loaded as FULL-ROW tiles (one 2.6MB transfer per tensor per
    group, 16KB contiguous runs) instead of two 1.3MB half-row transfers
  - fv/bv issued on the ACT HWDGE ring, m/fd/bd + stats on the SP ring
  - io_big bufs=2 to fit SBUF
Compute consumes [P, 2048] chunk views of the full-row tiles; per-chunk
op sequence and accumulator layout are identical to the baseline, so the
host assembly is unchanged.
"""

import sys

if "/opt/trn_rl_repo" not in sys.path:
    sys.path.insert(0, "/opt/trn_rl_repo")

import numpy as np

import concourse.bass as bass
import concourse.tile as tile
from concourse import mybir
from concourse.bass_utils import run_bass_kernel_spmd

N_CORES = 8
B_FULL = 256
B_SHARD = B_FULL // N_CORES  # 32
C = 20
RES = 64
SPATIAL = RES * RES          # 4096
ROWS = B_SHARD * C           # 640
P = 128
GROUPS = ROWS // P           # 5
CHUNK = 1024
NCHUNK = SPATIAL // CHUNK    # 2
EPS = 1e-6

F32 = mybir.dt.float32


def _build_program(repeat: int = 1) -> bass.Bass:
    nc = bass.Bass()

    pk = nc.declare_dram_parameter("packed", [ROWS, NCHUNK, 7, CHUNK], F32, isOutput=False)
    loc = nc.declare_dram_parameter("loc_const", [NCHUNK + 1, CHUNK], F32, isOutput=False)
    stats = nc.declare_dram_parameter("stats", [ROWS, NCHUNK * 8], F32, isOutput=True)

    MULT = mybir.AluOpType.mult

    with tile.TileContext(nc) as tc:
        with (
            tc.tile_pool(name="singles", bufs=1) as singles,
            tc.tile_pool(name="io", bufs=6) as io,
            tc.tile_pool(name="work", bufs=1) as work,
            tc.tile_pool(name="acc", bufs=4) as accp,
            tc.tile_pool(name="psum", bufs=2, space="PSUM") as psum,
        ):
            locx0 = singles.tile([P, CHUNK], F32, tag="locx0")
            nc.gpsimd.dma_start(out=locx0, in_=loc[0:1, :].to_broadcast([P, CHUNK]))
            locy = singles.tile([P, CHUNK], F32)
            nc.gpsimd.dma_start(out=locy, in_=loc[NCHUNK:NCHUNK + 1, :].to_broadcast([P, CHUNK]))

            for g in range(GROUPS * repeat):
                g = g % GROUPS
                r0 = g * P
                acc_t = accp.tile([P, NCHUNK, 8], F32, tag="acc")

                for ci in range(NCHUNK):
                    pk_t = io.tile([P, 7, CHUNK], F32, tag="pk")
                    eng = nc.sync if ci % 2 == 0 else nc.scalar
                    eng.dma_start(out=pk_t, in_=pk[r0:r0 + P, ci, :, :])
                    m_t = pk_t[:, 0, :]
                    fd_t = pk_t[:, 1, :]
                    bd_t = pk_t[:, 2, :]

                    t_f = work.tile([P, CHUNK], F32, tag="t_f")
                    t_b = work.tile([P, CHUNK], F32, tag="t_b")
                    pscr = psum.tile([P, CHUNK], F32, tag="pscr")

                    # msum on ScalarE (activation w/ accumulate; out -> PSUM)
                    nc.scalar.activation(
                        out=pscr,
                        in_=m_t,
                        func=mybir.ActivationFunctionType.Copy,
                        accum_out=acc_t[:, ci, 6:7],
                    )

                    # products as scalar_tensor_tensor: tensor-scalar family
                    # (2x-capable fp32 uops), vs tensor_tensor's 1x cap
                    nc.vector.scalar_tensor_tensor(
                        out=t_f, in0=m_t, scalar=1.0, in1=fd_t,
                        op0=MULT, op1=MULT)

                    def fused_dot(in0, in1, out, qi, scale):
                        # out = (in0 * scale) * in1 ; accum = sum(out)
                        nc.vector.scalar_tensor_tensor(
                            out=out,
                            in0=in0,
                            scalar=scale,
                            in1=in1,
                            op0=MULT,
                            op1=MULT,
                            accum_out=acc_t[:, ci, qi:qi + 1],
                        )

                    # front/back votes: sum(t * vec) * 64  (scalar folds res=64)
                    fused_dot(pk_t[:, 3, :], t_f, t_b, 0, 64.0)   # t_b garbage yet
                    fused_dot(pk_t[:, 4, :], t_f, t_f, 1, 64.0)   # last t_f read
                    nc.vector.scalar_tensor_tensor(
                        out=t_b, in0=m_t, scalar=1.0, in1=bd_t,
                        op0=MULT, op1=MULT)
                    fused_dot(pk_t[:, 5, :], t_b, t_f, 2, 64.0)   # t_f dead
                    fused_dot(pk_t[:, 6, :], t_b, t_b, 3, 64.0)   # last t_b read
                    # mask-location moments
                    fused_dot(m_t, locx0, t_f, 4, 1.0)            # t_f dead
                    fused_dot(m_t, locy, t_b, 5, 1.0)             # t_b dead

                nc.sync.dma_start(
                    out=stats[r0:r0 + P, :],
                    in_=acc_t.rearrange("p a b -> p (a b)"),
                )

    from concourse.library_overlay import lower_extended_insts
    lower_extended_insts(nc)
    _legalize_waits(nc)
    return nc


def _legalize_waits(nc) -> None:
    for f in nc.m.functions:
        for blk in f.blocks:
            insts = blk.instructions
            new_list = []
            changed = False
            for ins in insts:
                si = getattr(ins, "sync_info", None)
                ow = list(si.on_wait) if (si is not None and si.on_wait) else []
                cap = 2 if isinstance(ins, mybir.InstEventSemaphore) else 1
                if len(ow) > cap:
                    excess, keep = ow[:-cap], ow[-cap:]
                    for j in range(0, len(excess), 2):
                        ev = mybir.InstEventSemaphore(
                            name=f"{ins.name}-lw{j}", ins=[], outs=[]
                        )
                        ev.engine = ins.engine
                        ev.sync_info = mybir.SyncInfo(
                            on_wait=excess[j:j + 2], on_update=[]
                        )
                        new_list.append(ev)
                    ins.sync_info = mybir.SyncInfo(
                        on_wait=keep,
                        on_update=list(si.on_update) if si.on_update else [],
                    )
                    changed = True
                new_list.append(ins)
            if changed:
                blk.instructions.clear()
                blk.instructions.extend(new_list)


_PROGRAM_CACHE: dict = {}


def _get_program() -> bass.Bass:
    if "nc" not in _PROGRAM_CACHE:
        _PROGRAM_CACHE["nc"] = _build_program()
    return _PROGRAM_CACHE["nc"]


def _make_in_maps(front_vec, front_dis, back_vec, back_dis, ske_mask):
    fv = np.ascontiguousarray(np.asarray(front_vec, dtype=np.float32))
    fd = np.ascontiguousarray(np.asarray(front_dis, dtype=np.float32))
    bv = np.ascontiguousarray(np.asarray(back_vec, dtype=np.float32))
    bd = np.ascontiguousarray(np.asarray(back_dis, dtype=np.float32))
    m = np.ascontiguousarray(np.asarray(ske_mask, dtype=np.float32))

    p = np.arange(SPATIAL)
    locx_full = (p // RES).astype(np.float32).reshape(NCHUNK, CHUNK)
    locy_row = (p[:CHUNK] % RES).astype(np.float32)
    loc_const = np.ascontiguousarray(
        np.concatenate([locx_full, locy_row[None, :]], axis=0)
    )

    in_maps = []
    for i in range(N_CORES):
        sl = slice(i * B_SHARD, (i + 1) * B_SHARD)
        mm = m[sl].reshape(ROWS, NCHUNK, CHUNK)
        fdd = fd[sl].reshape(ROWS, NCHUNK, CHUNK)
        bdd = bd[sl].reshape(ROWS, NCHUNK, CHUNK)
        fvv = fv[sl].reshape(ROWS, NCHUNK, CHUNK, 2)
        bvv = bv[sl].reshape(ROWS, NCHUNK, CHUNK, 2)
        packed = np.empty((ROWS, NCHUNK, 7, CHUNK), dtype=np.float32)
        packed[:, :, 0] = mm
        packed[:, :, 1] = fdd
        packed[:, :, 2] = bdd
        packed[:, :, 3] = fvv[..., 0]
        packed[:, :, 4] = fvv[..., 1]
        packed[:, :, 5] = bvv[..., 0]
        packed[:, :, 6] = bvv[..., 1]
        in_maps.append({"packed": packed, "loc_const": loc_const})
    return in_maps


def _assemble(stats: np.ndarray) -> np.ndarray:
    """stats: [B, 20, NCHUNK*8] raw accumulators -> kp [B, 21, 2]."""
    B = stats.shape[0]
    acc = stats.reshape(B, C, NCHUNK, 8).astype(np.float32)
    s = acc.sum(axis=2)
    for ci in range(1, NCHUNK):
        s[:, :, 4] += np.float32(ci * (CHUNK // RES)) * acc[:, :, ci, 6]
    msum = s[:, :, 6]
    r = np.float32(1.0) / (msum + np.float32(EPS))
    F_ = np.stack([(s[:, :, 0] + s[:, :, 4]) * r, (s[:, :, 1] + s[:, :, 5]) * r], -1)
    Bk = np.stack([(s[:, :, 2] + s[:, :, 4]) * r, (s[:, :, 3] + s[:, :, 5]) * r], -1)

    root_terms = np.where(
        (msum[:, ::4] != 0.0)[..., None], Bk[:, ::4], np.float32(0.0)
    )  # [B,5,2]
    kp0 = root_terms.sum(axis=1, dtype=np.float32) / np.float32(5.0)  # [B,2]

    Fg = F_.reshape(B, 5, 4, 2)
    Bg = Bk.reshape(B, 5, 4, 2)
    tail = np.stack(
        [
            Fg[:, :, 3],
            (Fg[:, :, 2] + Bg[:, :, 3]) * np.float32(0.5),
            (Fg[:, :, 1] + Bg[:, :, 2]) * np.float32(0.5),
            (Fg[:, :, 0] + Bg[:, :, 1]) * np.float32(0.5),
        ],
        axis=2,
    )  # [B,5,4,2]
    kp = np.concatenate([kp0[:, None], tail.reshape(B, 20, 2)], axis=1)
    return (kp * np.float32(4.0)).astype(np.float32)


def kernel(front_vec, front_dis, back_vec, back_dis, ske_mask) -> np.ndarray:
    in_maps = _make_in_maps(front_vec, front_dis, back_vec, back_dis, ske_mask)
    nc = _get_program()
    res = run_bass_kernel_spmd(nc, in_maps, list(range(N_CORES)))
    stats = np.stack([np.asarray(res.results[i]["stats"]) for i in range(N_CORES)])
    stats = stats.reshape(B_FULL, C, NCHUNK * 8)
    return _assemble(stats)
